# revision 22
# baseline (speedup 1.0000x reference)
"""Causal GQA attention on 8 TRN2 NeuronCores.

Problem: q [4096, 4096] = [bs*seq, 32 heads * 128], k/v [4096, 1024] =
[bs*seq, 8 kv heads * 128], causal softmax(q k^T / sqrt(128)) v with GQA
(4 query heads per kv head). f32 in/out.

Sharding: 8 cores = 2 batches x 4 head-groups. Each core owns one batch
and 8 query heads / 2 kv heads -- fully local, no collectives. Q and K are
handed to each core pre-permuted to [head_dim, head, seq] (host-side layout
marshalling in the shard step) so the contraction dim is already on
partitions; V stays [seq, d] as the PV matmul wants it.

Per-core algorithm (Python-unrolled, Tile-scheduled):
  - For each head h, 1024-wide query group g, key block j (128 keys):
    S^T[k, q] = K_j^T Q  (f32r matmuls at 1 cycle/row, contraction over d
    on partitions), causal mask add on the diagonal 128x128 subtile (DVE),
    exp via ScalarE (scale folded into the activation) emitting bf16 probs
    P^T straight from PSUM.
  - Once the diagonal P^T for query subtile s exists, one PSUM bank runs
    the whole PV accumulation chain for s: out[q, 0:128] = sum_j P^T_j V_j
    and out[q, 128] = softmax denominator (ones column fused into V).
  - VectorE normalizes rows by the reciprocal denominator; stores go out
    on fresh HWDGE queues.

Walrus sync-wait limits (1 slot on DMA descriptors and LDWEIGHTS): all
loads land upfront in fresh buffers (zero waits), tiny PE warmup matmuls
absorb the DMA/DVE semaphores into PE's vector clock, and the mask lives
in a raw pre-Tile preamble so it is dependency-free.

No max-subtraction softmax: logits are ~N(0,1) after scale, exp stays in
range; masked entries get -1e9 so exp -> 0 exactly.
"""

import numpy as np

P = 128          # partitions / head_dim / key block
SEQ = 2048       # per-core sequence length
H = 8            # query heads per core
KV = 2           # kv heads per core
D = 128          # head dim
NB = SEQ // P    # 16 seq blocks
G = 2            # query groups per head
GW = SEQ // G    # 1024 query-group width
GB = GW // P     # 8 query subtiles per group
SCALE = float(D) ** -0.5
NEG = -1.0e9

_NC = None


def _build_nc():
    import concourse.bass as bass
    import concourse.bacc as bacc
    import concourse.mybir as mybir
    import concourse.tile as tile
    from contextlib import ExitStack

    f32 = mybir.dt.float32
    f32r = mybir.dt.float32r
    bf16 = mybir.dt.bfloat16
    Exp = mybir.ActivationFunctionType.Exp

    nc = bacc.Bacc()
    qT_ext = nc.declare_dram_parameter("qT", [P, H, SEQ], f32r, isOutput=False)
    kT_ext = nc.declare_dram_parameter("kT", [P, KV, SEQ], f32r, isOutput=False)
    v_ext = nc.declare_dram_parameter("v", [SEQ, KV * D], f32, isOutput=False)
    o_ext = nc.declare_dram_parameter("out", [SEQ, H * D], f32, isOutput=True)

    vd = v_ext.rearrange("(i p) c -> p i c", p=P)
    od = o_ext.rearrange("(i p) c -> p i c", p=P)

    def qk_chunks(a):
        """Bank-aligned <=512-wide chunks covering [a, GW); pieces narrower
        than 256 are widened leftward (within their bank) so f32r matmuls
        stay at the 1 cycle/row rate. Widened cols hold garbage never read."""
        cs = []
        c = a
        while c < GW:
            nxt = min(GW, (c // 512 + 1) * 512)
            c0 = c
            if nxt - c0 < 256:
                c0 = max(nxt - 256, (nxt - 1) // 512 * 512)
            cs.append((c0, nxt))
            c = nxt
        return cs

    # Causal mask for S^T diag subtiles, built in a raw pre-Tile preamble
    # (gpsimd in-order + all-engine barrier => dependency-free inside Tile).
    maskT = nc.alloc_sbuf_tensor("maskT", [P, P], f32).ap()
    nc.gpsimd.affine_select(
        out=maskT,
        in_=nc.const_aps.tensor(0.0, (P, P)),
        compare_op=mybir.AluOpType.is_ge,
        fill=NEG,
        base=0,
        pattern=[[1, P]],   # keep where q_local - k_local >= 0
        channel_multiplier=-1,
    )
    nc.all_engine_barrier()

    with ExitStack() as ctx:
        tc = ctx.enter_context(tile.TileContext(nc))
        singles = ctx.enter_context(tc.tile_pool(name="singles", bufs=1))
        pt_pool = ctx.enter_context(tc.tile_pool(name="pt", bufs=22))
        ob_pool = ctx.enter_context(tc.tile_pool(name="ob", bufs=2))
        r_pool = ctx.enter_context(tc.tile_pool(name="r", bufs=8))
        ps_st = ctx.enter_context(tc.tile_pool(name="ps_st", bufs=3, space="PSUM"))
        ps_pv = ctx.enter_context(tc.tile_pool(name="ps_pv", bufs=2, space="PSUM"))

        # ---- upfront loads, each into a fresh buffer on a fresh queue ----
        # The pieces head 0 / group 0 needs come first and are small, so
        # compute starts as soon as ~1.5MB has landed.
        kt = singles.tile([P, KV, SEQ], f32r)      # [d, kv, key]
        qt = singles.tile([P, H, SEQ], f32r)       # [d, head, query]
        v_nat = singles.tile([P, NB, KV * D], f32)
        for c in range(0, GW, 256):
            nc.gpsimd.dma_start(out=kt[:, 0, c:c + 256],
                                in_=kT_ext.ap()[:, 0, c:c + 256])
            nc.gpsimd.dma_start(out=qt[:, 0, c:c + 256],
                                in_=qT_ext.ap()[:, 0, c:c + 256])
        nc.gpsimd.dma_start(out=v_nat[:, 0:GB // 2, 0:D], in_=vd[:, 0:GB // 2, 0:D])
        nc.gpsimd.dma_start(out=v_nat[:, GB // 2:GB, 0:D], in_=vd[:, GB // 2:GB, 0:D])
        nc.gpsimd.dma_start(out=kt[:, 0, GW:], in_=kT_ext.ap()[:, 0, GW:])
        nc.gpsimd.dma_start(out=qt[:, 0, GW:], in_=qT_ext.ap()[:, 0, GW:])
        nc.gpsimd.dma_start(out=v_nat[:, GB:, 0:D], in_=vd[:, GB:, 0:D])
        nc.gpsimd.dma_start(out=v_nat[:, :, D:], in_=vd[:, :, D:])
        nc.gpsimd.dma_start(out=kt[:, 1:2, :], in_=kT_ext.ap()[:, 1:2, :])
        for i in range(1, H):
            nc.gpsimd.dma_start(out=qt[:, i:i + 1, :], in_=qT_ext.ap()[:, i:i + 1, :])

        vones = singles.tile([P, NB, KV, D + 1], bf16)  # [k, block, kv, d|1]
        nc.vector.tensor_copy(out=vones[:, 0:GB // 2, 0, 0:D],
                              in_=v_nat[:, 0:GB // 2, 0:D])
        nc.vector.tensor_copy(out=vones[:, GB // 2:GB, 0, 0:D],
                              in_=v_nat[:, GB // 2:GB, 0:D])
        nc.vector.tensor_copy(out=vones[:, GB:, 0, 0:D], in_=v_nat[:, GB:, 0:D])
        nc.vector.tensor_copy(out=vones[:, :, 1, 0:D], in_=v_nat[:, :, D:2 * D])
        nc.vector.memset(vones[:, :, :, D:D + 1], 1.0)

        # ---- PE warmups: absorb every load's semaphore into PE's clock so
        # real matmuls never carry a second (DMA/DVE) wait. Outputs unread.
        def warm(ap):
            # f32r forbids tiny matmuls; bitcast to f16 (values unread).
            if ap.dtype == f32r:
                ap = ap.bitcast(mybir.dt.float16)
            n = ap.shape[-1]
            wm = ps_pv.tile([2, 2], f32, tag="pvacc", name="wm")
            nc.tensor.matmul(wm[:n, :n], lhsT=ap, rhs=ap, start=True, stop=True)

        for c in range(0, GW, 256):
            warm(kt[:, 0, c:c + 1])
            warm(qt[:, 0, c:c + 1])
        warm(vones[:, 0, 0, 0:1])
        actwarm = pt_pool.tile([P, P], bf16, tag="actwarm", name="actwarm")
        nc.scalar.activation(out=actwarm, in_=maskT, func=Exp, scale=SCALE)

        # ---- one global software pipeline over (head, group, key-block) ----
        # QK (+diag mask) runs LA units ahead of exp/PV across head and group
        # boundaries, so neither PE nor ScalarE drains at a seam. Warmups for
        # late-loaded DMA pieces ride in the QK stream just before first use.
        units = [(h, g, j) for h in range(H) for g in range(G)
                 for j in range(GB * (g + 1))]
        LA = 2  # matches ps_st bufs=3: slots u..u+2 live
        sts = {}
        pts = {}
        o_sbs = {}

        def emit_qk(u):
            h, g, j = units[u]
            kvh = h // (H // KV)
            if g == 0 and j == 0:  # first touch of this head's q slice
                if h > 0:
                    warm(qt[:, h, 0:1])
                if h == H // KV:
                    warm(kt[:, 1, 0:1])
                    warm(vones[:, 0, 1, 0:1])
            if h == 0 and g == 1 and j == 0:  # second halves of kt0/qt0/v-kv0
                warm(qt[:, 0, GW:GW + 1])
                warm(kt[:, 0, GW:GW + 1])
                warm(vones[:, GB, 0, 0:1])
            s0 = max(0, j - GB * g)
            st = ps_st.tile([P, GW], f32, name="st")
            for (c0, c1) in qk_chunks(s0 * P):
                nc.tensor.matmul(
                    st[:, c0:c1],
                    lhsT=kt[:, kvh, j * P:(j + 1) * P],
                    rhs=qt[:, h, g * GW + c0:g * GW + c1],
                    start=True,
                    stop=True,
                )
            if j >= GB * g:  # diagonal-band block: mask its diag subtile
                nc.vector.tensor_add(
                    out=st[:, s0 * P:(s0 + 1) * P],
                    in0=st[:, s0 * P:(s0 + 1) * P],
                    in1=maskT,
                )
            sts[u] = st

        for u in range(min(LA, len(units))):
            emit_qk(u)
        for u in range(len(units)):
            if u + LA < len(units):
                emit_qk(u + LA)
            h, g, j = units[u]
            kvh = h // (H // KV)
            if g == 0 and j == 0:
                o_sbs[h] = ob_pool.tile([P, NB, D], f32, name="o_sb")
                pts[h] = {}
            s0 = max(0, j - GB * g)
            pt = pt_pool.tile([P, GW], bf16, name="pt")
            nc.scalar.activation(
                out=pt[:, s0 * P:], in_=sts.pop(u)[:, s0 * P:],
                func=Exp, scale=SCALE
            )
            pts[h].setdefault(g, []).append(pt)
            if j >= GB * g:
                s = j - GB * g
                i_glob = g * GB + s
                acc = ps_pv.tile([P, D + 1], f32, name="pvacc")
                for j2 in range(GB * g + s + 1):
                    nc.tensor.matmul(
                        acc,
                        lhsT=pts[h][g][j2][:, s * P:(s + 1) * P],
                        rhs=vones[:, j2, kvh, :],
                        start=(j2 == 0),
                        stop=(j2 == GB * g + s),
                    )
                r = r_pool.tile([P, 1], f32, name="r")
                nc.vector.reciprocal(r, acc[:, D:D + 1])
                nc.vector.tensor_scalar_mul(
                    o_sbs[h][:, i_glob, :], acc[:, 0:D], r
                )
            if j >= GB * g and (j - GB * g) % 4 == 3:
                # store the 4 just-normalized subtiles (keeps the final
                # store small so the kernel tail is short)
                i0 = g * GB + (j - GB * g) - 3
                nc.sync.dma_start(
                    out=od[:, i0:i0 + 4, h * D:(h + 1) * D],
                    in_=o_sbs[h][:, i0:i0 + 4, :],
                )

    nc.compile()
    return nc


def _get_nc():
    global _NC
    if _NC is None:
        _NC = _build_nc()
    return _NC


def _shard_inputs(q, k, v):
    in_maps = []
    for c in range(8):
        b, hg = divmod(c, 4)
        rs = slice(b * SEQ, (b + 1) * SEQ)
        qs = q[rs, hg * 1024:(hg + 1) * 1024]    # [seq, 8*128]
        ks = k[rs, hg * 256:(hg + 1) * 256]      # [seq, 2*128]
        in_maps.append({
            "qT": np.ascontiguousarray(
                qs.reshape(SEQ, H, D).transpose(2, 1, 0)),
            "kT": np.ascontiguousarray(
                ks.reshape(SEQ, KV, D).transpose(2, 1, 0)),
            "v": np.ascontiguousarray(v[rs, hg * 256:(hg + 1) * 256]),
        })
    return in_maps


def _run(q, k, v, **spmd_kwargs):
    from concourse.bass_utils import run_bass_kernel_spmd

    nc = _get_nc()
    bkr = run_bass_kernel_spmd(nc, _shard_inputs(q, k, v),
                               core_ids=list(range(8)), **spmd_kwargs)
    out = np.empty((2 * SEQ, 32 * D), np.float32)
    for c in range(8):
        b, hg = divmod(c, 4)
        out[b * SEQ:(b + 1) * SEQ, hg * 1024:(hg + 1) * 1024] = \
            bkr.results[c]["out"]
    return out, bkr


def kernel(q, k, v, bs=2, seq_len=2048, **_ignored):
    q = np.asarray(q, dtype=np.float32)
    k = np.asarray(k, dtype=np.float32)
    v = np.asarray(v, dtype=np.float32)
    assert int(bs) == 2 and int(seq_len) == SEQ
    assert q.shape == (4096, 4096) and k.shape == (4096, 1024)
    out, _ = _run(q, k, v)
    return out


# revision 23
# speedup vs baseline: 1.0286x; 1.0286x over previous
"""Causal GQA attention on 8 TRN2 NeuronCores.

Problem: q [4096, 4096] = [bs*seq, 32 heads * 128], k/v [4096, 1024] =
[bs*seq, 8 kv heads * 128], causal softmax(q k^T / sqrt(128)) v with GQA
(4 query heads per kv head). f32 in/out.

Sharding: 8 cores = 2 batches x 4 head-groups. Each core owns one batch
and 8 query heads / 2 kv heads -- fully local, no collectives. Q and K are
handed to each core pre-permuted to [head_dim, head, seq] (host-side layout
marshalling in the shard step) so the contraction dim is already on
partitions; V stays [seq, d] as the PV matmul wants it.

Per-core algorithm (Python-unrolled, Tile-scheduled):
  - For each head h, 1024-wide query group g, key block j (128 keys):
    S^T[k, q] = K_j^T Q  (f32r matmuls at 1 cycle/row, contraction over d
    on partitions), causal mask add on the diagonal 128x128 subtile (DVE),
    exp via ScalarE (scale folded into the activation) emitting bf16 probs
    P^T straight from PSUM.
  - Once the diagonal P^T for query subtile s exists, one PSUM bank runs
    the whole PV accumulation chain for s: out[q, 0:128] = sum_j P^T_j V_j
    and out[q, 128] = softmax denominator (ones column fused into V).
  - VectorE normalizes rows by the reciprocal denominator; stores go out
    on fresh HWDGE queues.

Walrus sync-wait limits (1 slot on DMA descriptors and LDWEIGHTS): all
loads land upfront in fresh buffers (zero waits), tiny PE warmup matmuls
absorb the DMA/DVE semaphores into PE's vector clock, and the mask lives
in a raw pre-Tile preamble so it is dependency-free.

No max-subtraction softmax: logits are ~N(0,1) after scale, exp stays in
range; masked entries get -1e9 so exp -> 0 exactly.
"""

import numpy as np

P = 128          # partitions / head_dim / key block
SEQ = 2048       # per-core sequence length
H = 8            # query heads per core
KV = 2           # kv heads per core
D = 128          # head dim
NB = SEQ // P    # 16 seq blocks
G = 2            # query groups per head
GW = SEQ // G    # 1024 query-group width
GB = GW // P     # 8 query subtiles per group
SCALE = float(D) ** -0.5
NEG = -1.0e9

_NC = None


def _build_nc():
    import concourse.bass as bass
    import concourse.bacc as bacc
    import concourse.mybir as mybir
    import concourse.tile as tile
    from contextlib import ExitStack

    f32 = mybir.dt.float32
    f32r = mybir.dt.float32r
    bf16 = mybir.dt.bfloat16
    Exp = mybir.ActivationFunctionType.Exp

    nc = bacc.Bacc()
    qT_ext = nc.declare_dram_parameter("qT", [P, H, SEQ], bf16, isOutput=False)
    kT_ext = nc.declare_dram_parameter("kT", [P, KV, SEQ], bf16, isOutput=False)
    v_ext = nc.declare_dram_parameter("v", [SEQ, KV * D], f32, isOutput=False)
    o_ext = nc.declare_dram_parameter("out", [SEQ, H * D], f32, isOutput=True)

    vd = v_ext.rearrange("(i p) c -> p i c", p=P)
    od = o_ext.rearrange("(i p) c -> p i c", p=P)

    def qk_chunks(a):
        """Bank-aligned <=512-wide chunks covering [a, GW); pieces narrower
        than 256 are widened leftward (within their bank) so f32r matmuls
        stay at the 1 cycle/row rate. Widened cols hold garbage never read."""
        cs = []
        c = a
        while c < GW:
            nxt = min(GW, (c // 512 + 1) * 512)
            c0 = c
            if nxt - c0 < 256:
                c0 = max(nxt - 256, (nxt - 1) // 512 * 512)
            cs.append((c0, nxt))
            c = nxt
        return cs

    # Causal mask for S^T diag subtiles, built in a raw pre-Tile preamble
    # (gpsimd in-order + all-engine barrier => dependency-free inside Tile).
    maskT = nc.alloc_sbuf_tensor("maskT", [P, P], f32).ap()
    nc.gpsimd.affine_select(
        out=maskT,
        in_=nc.const_aps.tensor(0.0, (P, P)),
        compare_op=mybir.AluOpType.is_ge,
        fill=NEG,
        base=0,
        pattern=[[1, P]],   # keep where q_local - k_local >= 0
        channel_multiplier=-1,
    )
    nc.all_engine_barrier()

    with ExitStack() as ctx:
        tc = ctx.enter_context(tile.TileContext(nc))
        singles = ctx.enter_context(tc.tile_pool(name="singles", bufs=1))
        pt_pool = ctx.enter_context(tc.tile_pool(name="pt", bufs=22))
        ob_pool = ctx.enter_context(tc.tile_pool(name="ob", bufs=2))
        r_pool = ctx.enter_context(tc.tile_pool(name="r", bufs=8))
        ps_st = ctx.enter_context(tc.tile_pool(name="ps_st", bufs=3, space="PSUM"))
        ps_pv = ctx.enter_context(tc.tile_pool(name="ps_pv", bufs=2, space="PSUM"))

        # ---- upfront loads, each into a fresh buffer on a fresh queue ----
        # The pieces head 0 / group 0 needs come first and are small, so
        # compute starts as soon as ~1.5MB has landed.
        kt = singles.tile([P, KV, SEQ], bf16)      # [d, kv, key]
        qt = singles.tile([P, H, SEQ], bf16)       # [d, head, query]
        v_nat = singles.tile([P, NB, KV * D], f32)
        for c in range(0, GW, 256):
            nc.gpsimd.dma_start(out=kt[:, 0, c:c + 256],
                                in_=kT_ext.ap()[:, 0, c:c + 256])
            nc.gpsimd.dma_start(out=qt[:, 0, c:c + 256],
                                in_=qT_ext.ap()[:, 0, c:c + 256])
        nc.gpsimd.dma_start(out=v_nat[:, 0:GB // 2, 0:D], in_=vd[:, 0:GB // 2, 0:D])
        nc.gpsimd.dma_start(out=v_nat[:, GB // 2:GB, 0:D], in_=vd[:, GB // 2:GB, 0:D])
        nc.gpsimd.dma_start(out=kt[:, 0, GW:], in_=kT_ext.ap()[:, 0, GW:])
        nc.gpsimd.dma_start(out=qt[:, 0, GW:], in_=qT_ext.ap()[:, 0, GW:])
        nc.gpsimd.dma_start(out=v_nat[:, GB:, 0:D], in_=vd[:, GB:, 0:D])
        nc.gpsimd.dma_start(out=v_nat[:, :, D:], in_=vd[:, :, D:])
        nc.gpsimd.dma_start(out=kt[:, 1:2, :], in_=kT_ext.ap()[:, 1:2, :])
        for i in range(1, H):
            nc.gpsimd.dma_start(out=qt[:, i:i + 1, :], in_=qT_ext.ap()[:, i:i + 1, :])

        vones = singles.tile([P, NB, KV, D + 1], bf16)  # [k, block, kv, d|1]
        nc.vector.tensor_copy(out=vones[:, 0:GB // 2, 0, 0:D],
                              in_=v_nat[:, 0:GB // 2, 0:D])
        nc.vector.tensor_copy(out=vones[:, GB // 2:GB, 0, 0:D],
                              in_=v_nat[:, GB // 2:GB, 0:D])
        nc.vector.tensor_copy(out=vones[:, GB:, 0, 0:D], in_=v_nat[:, GB:, 0:D])
        nc.vector.tensor_copy(out=vones[:, :, 1, 0:D], in_=v_nat[:, :, D:2 * D])
        nc.vector.memset(vones[:, :, :, D:D + 1], 1.0)

        # ---- PE warmups: absorb every load's semaphore into PE's clock so
        # real matmuls never carry a second (DMA/DVE) wait. Outputs unread.
        def warm(ap):
            # f32r forbids tiny matmuls; bitcast to f16 (values unread).
            if ap.dtype == f32r:
                ap = ap.bitcast(mybir.dt.float16)
            n = ap.shape[-1]
            wm = ps_pv.tile([2, 2], f32, tag="pvacc", name="wm")
            nc.tensor.matmul(wm[:n, :n], lhsT=ap, rhs=ap, start=True, stop=True)

        for c in range(0, GW, 256):
            warm(kt[:, 0, c:c + 1])
            warm(qt[:, 0, c:c + 1])
        warm(vones[:, 0, 0, 0:1])
        actwarm = pt_pool.tile([P, P], bf16, tag="actwarm", name="actwarm")
        nc.scalar.activation(out=actwarm, in_=maskT, func=Exp, scale=SCALE)

        # ---- one global software pipeline over (head, group, key-block) ----
        # QK (+diag mask) runs LA units ahead of exp/PV across head and group
        # boundaries, so neither PE nor ScalarE drains at a seam. Warmups for
        # late-loaded DMA pieces ride in the QK stream just before first use.
        units = [(h, g, j) for h in range(H) for g in range(G)
                 for j in range(GB * (g + 1))]
        LA = 2  # matches ps_st bufs=3: slots u..u+2 live
        sts = {}
        pts = {}
        o_sbs = {}

        def emit_qk(u):
            h, g, j = units[u]
            kvh = h // (H // KV)
            if g == 0 and j == 0:  # first touch of this head's q slice
                if h > 0:
                    warm(qt[:, h, 0:1])
                if h == H // KV:
                    warm(kt[:, 1, 0:1])
                    warm(vones[:, 0, 1, 0:1])
            if h == 0 and g == 1 and j == 0:  # second halves of kt0/qt0/v-kv0
                warm(qt[:, 0, GW:GW + 1])
                warm(kt[:, 0, GW:GW + 1])
                warm(vones[:, GB, 0, 0:1])
            s0 = max(0, j - GB * g)
            st = ps_st.tile([P, GW], f32, name="st")
            for (c0, c1) in qk_chunks(s0 * P):
                nc.tensor.matmul(
                    st[:, c0:c1],
                    lhsT=kt[:, kvh, j * P:(j + 1) * P],
                    rhs=qt[:, h, g * GW + c0:g * GW + c1],
                    start=True,
                    stop=True,
                )
            if j >= GB * g:  # diagonal-band block: mask its diag subtile
                nc.vector.tensor_add(
                    out=st[:, s0 * P:(s0 + 1) * P],
                    in0=st[:, s0 * P:(s0 + 1) * P],
                    in1=maskT,
                )
            sts[u] = st

        for u in range(min(LA, len(units))):
            emit_qk(u)
        for u in range(len(units)):
            if u + LA < len(units):
                emit_qk(u + LA)
            h, g, j = units[u]
            kvh = h // (H // KV)
            if g == 0 and j == 0:
                o_sbs[h] = ob_pool.tile([P, NB, D], f32, name="o_sb")
                pts[h] = {}
            s0 = max(0, j - GB * g)
            pt = pt_pool.tile([P, GW], bf16, name="pt")
            nc.scalar.activation(
                out=pt[:, s0 * P:], in_=sts.pop(u)[:, s0 * P:],
                func=Exp, scale=SCALE
            )
            pts[h].setdefault(g, []).append(pt)
            if j >= GB * g:
                s = j - GB * g
                i_glob = g * GB + s
                acc = ps_pv.tile([P, D + 1], f32, name="pvacc")
                for j2 in range(GB * g + s + 1):
                    nc.tensor.matmul(
                        acc,
                        lhsT=pts[h][g][j2][:, s * P:(s + 1) * P],
                        rhs=vones[:, j2, kvh, :],
                        start=(j2 == 0),
                        stop=(j2 == GB * g + s),
                    )
                r = r_pool.tile([P, 1], f32, name="r")
                nc.vector.reciprocal(r, acc[:, D:D + 1])
                nc.vector.tensor_scalar_mul(
                    o_sbs[h][:, i_glob, :], acc[:, 0:D], r
                )
            if j >= GB * g and (j - GB * g) % 4 == 3:
                # store the 4 just-normalized subtiles (keeps the final
                # store small so the kernel tail is short)
                i0 = g * GB + (j - GB * g) - 3
                nc.sync.dma_start(
                    out=od[:, i0:i0 + 4, h * D:(h + 1) * D],
                    in_=o_sbs[h][:, i0:i0 + 4, :],
                )

    nc.compile()
    return nc


def _get_nc():
    global _NC
    if _NC is None:
        _NC = _build_nc()
    return _NC


def _shard_inputs(q, k, v):
    in_maps = []
    for c in range(8):
        b, hg = divmod(c, 4)
        rs = slice(b * SEQ, (b + 1) * SEQ)
        qs = q[rs, hg * 1024:(hg + 1) * 1024]    # [seq, 8*128]
        ks = k[rs, hg * 256:(hg + 1) * 256]      # [seq, 2*128]
        import ml_dtypes
        in_maps.append({
            "qT": np.ascontiguousarray(
                qs.reshape(SEQ, H, D).transpose(2, 1, 0)
            ).astype(ml_dtypes.bfloat16),
            "kT": np.ascontiguousarray(
                ks.reshape(SEQ, KV, D).transpose(2, 1, 0)
            ).astype(ml_dtypes.bfloat16),
            "v": np.ascontiguousarray(v[rs, hg * 256:(hg + 1) * 256]),
        })
    return in_maps


def _run(q, k, v, **spmd_kwargs):
    from concourse.bass_utils import run_bass_kernel_spmd

    nc = _get_nc()
    bkr = run_bass_kernel_spmd(nc, _shard_inputs(q, k, v),
                               core_ids=list(range(8)), **spmd_kwargs)
    out = np.empty((2 * SEQ, 32 * D), np.float32)
    for c in range(8):
        b, hg = divmod(c, 4)
        out[b * SEQ:(b + 1) * SEQ, hg * 1024:(hg + 1) * 1024] = \
            bkr.results[c]["out"]
    return out, bkr


def kernel(q, k, v, bs=2, seq_len=2048, **_ignored):
    q = np.asarray(q, dtype=np.float32)
    k = np.asarray(k, dtype=np.float32)
    v = np.asarray(v, dtype=np.float32)
    assert int(bs) == 2 and int(seq_len) == SEQ
    assert q.shape == (4096, 4096) and k.shape == (4096, 1024)
    out, _ = _run(q, k, v)
    return out
